# revision 30
# baseline (speedup 1.0000x reference)
"""Linear-CKA map kernel for Trainium2 (8 NeuronCores, SPMD, no collectives).

Math: for activations X[l] ([B, D] per layer), the reference computes
Gram matrices G_l = X_l X_l^T, double-centers them (Gc = H G H), and
hsic[i,j] = <Gc_i, Gc_j>, cka = hsic / sqrt(diag outer).

We use the expansion (H idempotent, G symmetric):
    hsic_ij = S_ij - (2/B) * T_ij + u_i u_j / B^2
      S_ij = <G_i, G_j>
      rowsum_l[b] = sum_c G_l[b, c] = X_l[b, :] . s_l,  s_l = sum_b X_l[b, :]
      T_ij = sum_b rowsum_i[b] rowsum_j[b]
      u_l  = s_l . s_l

Sharding: the Gram is symmetric, so only its block upper triangle is
needed.  With 16x16 blocks of [128, 128], core k computes the cyclic
cover blocks (bi, (bi + t) mod 16), t = 0..8, for its two block rows
bi in {2k, 2k+1} -- a perfectly uniform SPMD program (18 blocks per
core; every unordered block pair is covered once, except the t=8
antipodal blocks which two cores split).  Weight classes: t=0 diag
blocks count once, t=1..7 twice, t=8 once; the three classes accumulate
into separate [24,24] PSUM groups and the host combines g0 + 2*g1 + g2.

Per-core input: xr = X^T columns [2k*128 : 2k*128+1280] (mod B), in
fp8-e4m3 (CKA is a normalized statistic, so input quantization noise
stays ~1e-4 in the output) -- both Gram matmul operands come from this
one 63 MB slice, read exactly once, and the matmuls run in DoubleRow
mode (256-deep contraction, 2 fp8 MACs/cell/cycle).  S itself reduces
on TensorE: the PSUM->SBUF copies interleave layers ([b, c, layer] in
bf16), then [128b, 96] x [128b, 96] matmuls (4 Gram columns per call)
accumulate S in PSUM; the host keeps the diagonal [24,24] blocks.
Partial S is the only device output, summed on the host -- no device
collective at all.  The O(L*B*D) row-sum statistics T and u are
computed on the host (0.02% of the FLOPs).
"""

import numpy as np
import ml_dtypes

L, B, D = 24, 2048, 2048
NCORES = 8
P = 128
NBLK = B // P               # 16 block rows/cols
JT = D // (2 * P)           # 8 double-row contraction tiles (256 deep each)
JG = 2                      # j-tiles fetched per rhs DMA
NT = 9                      # cyclic block offsets t = 0..8 per block row
NR = 2                      # block rows per core
NXC = NT + NR - 1           # 10 column chunks staged per core
WC = NT * P                 # 1152 Gram columns per block row

_NC_CACHE = {}


def _build():
    if "nc" in _NC_CACHE:
        return _NC_CACHE["nc"]
    import concourse.bass as bass
    from concourse import bacc, mybir, tile

    f32 = mybir.dt.float32
    bf16 = mybir.dt.bfloat16
    fp8 = mybir.dt.float8e4
    DR = mybir.MatmulPerfMode.DoubleRow
    Act = mybir.ActivationFunctionType

    nc = bacc.Bacc("TRN2", target_bir_lowering=False, debug=False)

    # xr is host-packed to exactly match the SBUF staging tiles: one fully
    # linear [P, JG, 2, 1280] block per (layer, jg) DMA
    xr = nc.dram_tensor(
        "xr", [L, JT // JG, P, JG, 2, NXC * P], fp8, kind="ExternalInput"
    )
    s_out = nc.dram_tensor("s_out", [3, 4 * L, 4 * L], f32, kind="ExternalOutput")

    with tile.TileContext(nc) as tc:
        with (
            tc.tile_pool(name="gb", bufs=1) as gbpool,
            tc.tile_pool(name="rt", bufs=5) as rtpool,
            tc.tile_pool(name="small", bufs=1) as smallpool,
            tc.tile_pool(name="psum", bufs=2, space=bass.MemorySpace.PSUM) as psumpool,
            tc.tile_pool(name="psumS", bufs=1, space=bass.MemorySpace.PSUM) as psSpool,
        ):
            # persistent SBUF: interleaved Gram store [b, c, layer] per row
            gbig = [
                gbpool.tile([P, WC, L], bf16, tag=f"Gbig{r}", name=f"Gbig{r}")
                for r in range(NR)
            ]
            # NOTE: matmul start=True pending-zeroes its whole PSUM bank, so
            # each bank may host only ONE accumulation group at a time (the
            # 512/512/128 Gram split below is bank-aligned for this reason,
            # and the S classes accumulate sequentially with copies between).
            QW = 4 * L          # S-stage batches 4 Gram columns per matmul
            ptS = psSpool.tile([P, 3 * QW], f32, tag="psS")

            for l in range(L):
                pt = [
                    psumpool.tile([P, WC], f32, tag="pm", name=f"pm{r}")
                    for r in range(NR)
                ]
                for jg in range(JT // JG):
                    rt = rtpool.tile([P, JG, 2, NXC * P], fp8, tag="rt")
                    nc.sync.dma_start(rt[:, :, :, :], xr[l, jg])
                    for jj in range(JG):
                        j = jg * JG + jj
                        for r in range(NR):
                            lhs = rt[:, jj, :, r * P : (r + 1) * P]
                            # 9 cyclic blocks = contiguous 1152 rhs columns,
                            # split 512/512/128 on PSUM bank boundaries
                            for c0, cw in ((0, 512), (512, 512), (1024, 128)):
                                nc.tensor.matmul(
                                    pt[r][:, c0 : c0 + cw],
                                    lhsT=lhs,
                                    rhs=rt[:, jj, :, r * P + c0 : r * P + c0 + cw],
                                    start=(j == 0),
                                    stop=(j == JT - 1),
                                    perf_mode=DR,
                                )
                for r in range(NR):
                    nc.scalar.copy(gbig[r][:, :, l], pt[r][:, :])

            # S-stage on TensorE: [128b, 96] x [128b, 96] matmuls, 4 Gram
            # columns per call (gbig's [c, l] layout is contiguous, so 4
            # columns = one flat 96-wide operand).  Only the 4 diagonal
            # [24, 24] blocks of each [96, 96] product matter; the host
            # discards the cross-column junk.  Weight classes accumulate
            # SEQUENTIALLY (one live group in the shared PSUM bank) and are
            # copied out before the next class's start=True re-marks the bank.
            sall = smallpool.tile([P, 3 * QW], f32, tag="sall")
            cls_bounds = {0: (0, P), 1: (P, 8 * P), 2: (8 * P, WC)}
            for cls in range(3):
                lo, hi = cls_bounds[cls]
                for r in range(NR):
                    for c in range(lo, hi, 4):
                        nc.tensor.matmul(
                            ptS[0:QW, cls * QW : (cls + 1) * QW],
                            lhsT=gbig[r][:, c : c + 4, :],
                            rhs=gbig[r][:, c : c + 4, :],
                            start=(r == 0 and c == lo),
                            stop=(r == NR - 1 and c == hi - 4),
                        )
                nc.scalar.copy(
                    sall[0:QW, cls * QW : (cls + 1) * QW],
                    ptS[0:QW, cls * QW : (cls + 1) * QW],
                )
            for g in range(3):
                nc.sync.dma_start(s_out[g], sall[0:QW, g * QW : (g + 1) * QW])

    nc.compile()
    _NC_CACHE["nc"] = nc
    return nc


def _run(activations, trace=False):
    from concourse.bass_utils import run_bass_kernel_spmd

    x = np.asarray(activations, dtype=np.float32)
    assert x.shape == (L, B, D)
    xt_np = np.ascontiguousarray(x.transpose(0, 2, 1)).astype(ml_dtypes.float8_e4m3)
    s_star = xt_np.astype(np.float64).sum(axis=2)  # [L, D], exact sum of fp8 X

    in_maps = []
    for c in range(NCORES):
        lo = NR * c * P
        rolled = np.concatenate([xt_np[:, :, lo:], xt_np[:, :, :lo]], axis=2)[
            :, :, : NXC * P
        ]
        # pack to the SBUF staging layout: [L, jg, p, jj, i, n] so each
        # (layer, jg) DMA is one fully contiguous block
        packed = np.ascontiguousarray(
            rolled.reshape(L, JT // JG, JG, 2, P, NXC * P).transpose(0, 1, 4, 2, 3, 5)
        )
        in_maps.append({"xr": packed})
    nc = _build()
    try:
        res = run_bass_kernel_spmd(
            nc, in_maps, core_ids=list(range(NCORES)), trace=trace
        )
    except Exception:
        # transient NRT_EXEC_UNIT_UNRECOVERABLE device states have been
        # observed to clear on the next attempt
        import time

        time.sleep(5)
        res = run_bass_kernel_spmd(
            nc, in_maps, core_ids=list(range(NCORES)), trace=trace
        )

    S = np.zeros((L, L), dtype=np.float64)
    for c in range(NCORES):
        g = res.results[c]["s_out"].astype(np.float64)
        gd = [
            sum(g[i, d * L : (d + 1) * L, d * L : (d + 1) * L] for d in range(4))
            for i in range(3)
        ]
        S += gd[0] + 2.0 * gd[1] + gd[2]

    # row-sum statistics are O(L*B*D) -- computed host-side on the same
    # quantized values the device consumed
    xq = xt_np.astype(np.float32)                  # [L, D, B]
    rowsum = np.einsum("ldb,ld->lb", xq, s_star.astype(np.float32))
    T = np.einsum("ib,jb->ij", rowsum, rowsum, dtype=np.float64)
    u = np.einsum("ld,ld->l", s_star, s_star)
    hsic = S - (2.0 / B) * T + np.outer(u, u) / (B * B)
    norms = np.sqrt(np.diagonal(hsic))
    cka = hsic / (norms[:, None] * norms[None, :])
    return cka.astype(np.float32), res


def kernel(activations):
    cka, _ = _run(activations, trace=False)
    return cka


def run_traced(activations):
    return _run(activations, trace=True)


# revision 37
# speedup vs baseline: 1.0419x; 1.0419x over previous
"""Linear-CKA map kernel for Trainium2 (8 NeuronCores, SPMD, no collectives).

Math: for activations X[l] ([B, D] per layer), the reference computes
Gram matrices G_l = X_l X_l^T, double-centers them (Gc = H G H), and
hsic[i,j] = <Gc_i, Gc_j>, cka = hsic / sqrt(diag outer).

We use the expansion (H idempotent, G symmetric):
    hsic_ij = S_ij - (2/B) * T_ij + u_i u_j / B^2
      S_ij = <G_i, G_j>
      rowsum_l[b] = sum_c G_l[b, c] = X_l[b, :] . s_l,  s_l = sum_b X_l[b, :]
      T_ij = sum_b rowsum_i[b] rowsum_j[b]
      u_l  = s_l . s_l

Sharding: the Gram is symmetric, so only its block upper triangle is
needed.  With 16x16 blocks of [128, 128], core k computes the cyclic
cover blocks (bi, (bi + t) mod 16), t = 0..8, for its two block rows
bi in {2k, 2k+1} -- a perfectly uniform SPMD program (18 blocks per
core; every unordered block pair is covered once, except the t=8
antipodal blocks which two cores split).  Weight classes: t=0 diag
blocks count once, t=1..7 twice, t=8 once; the three classes accumulate
into separate [24,24] PSUM groups and the host combines g0 + 2*g1 + g2.

Per-core input: xr = X^T columns [2k*128 : 2k*128+1280] (mod B), in
fp8-e4m3 (CKA is a normalized statistic, so input quantization noise
stays ~1e-4 in the output) -- both Gram matmul operands come from this
one 63 MB slice, read exactly once, and the matmuls run in DoubleRow
mode (256-deep contraction, 2 fp8 MACs/cell/cycle).  S itself reduces
on TensorE: the PSUM->SBUF copies interleave layers ([b, c, layer] in
bf16), then [128b, 96] x [128b, 96] matmuls (4 Gram columns per call)
accumulate S in PSUM; the host keeps the diagonal [24,24] blocks.
Partial S is the only device output, summed on the host -- no device
collective at all.  The O(L*B*D) row-sum statistics T and u are
computed on the host (0.02% of the FLOPs).
"""

import numpy as np
import ml_dtypes

L, B, D = 24, 2048, 2048
NCORES = 8
P = 128
NBLK = B // P               # 16 block rows/cols
JT = D // (2 * P)           # 8 double-row contraction tiles (256 deep each)
JG = 2                      # j-tiles fetched per rhs DMA
NT = 9                      # cyclic block offsets t = 0..8 per block row
NR = 2                      # block rows per core
NXC = NT + NR - 1           # 10 column chunks staged per core
WC = NT * P                 # 1152 Gram columns per block row
LA = 9                      # layers whose intra-pairs reduce on idle VectorE
LB = L - LA                 # layers handled by the TensorE S-stage rhs
NPA = LA * (LA + 1) // 2    # 45 VectorE pairs

_NC_CACHE = {}


def _build():
    if "nc" in _NC_CACHE:
        return _NC_CACHE["nc"]
    import concourse.bass as bass
    from concourse import bacc, mybir, tile

    f32 = mybir.dt.float32
    bf16 = mybir.dt.bfloat16
    fp8 = mybir.dt.float8e4
    DR = mybir.MatmulPerfMode.DoubleRow
    Act = mybir.ActivationFunctionType
    Alu = mybir.AluOpType

    nc = bacc.Bacc("TRN2", target_bir_lowering=False, debug=False)

    # xr is host-packed to exactly match the SBUF staging tiles: one fully
    # linear [P, JG, 2, 1280] block per (layer, jg) DMA
    xr = nc.dram_tensor(
        "xr", [L, JT // JG, P, JG, 2, NXC * P], fp8, kind="ExternalInput"
    )
    s_out = nc.dram_tensor("s_out", [3, 4 * L, 4 * LB], f32, kind="ExternalOutput")
    a_out = nc.dram_tensor("a_out", [1, NR * 3 * NPA], f32, kind="ExternalOutput")

    with tile.TileContext(nc) as tc:
        with (
            tc.tile_pool(name="gb", bufs=1) as gbpool,
            tc.tile_pool(name="rt", bufs=5) as rtpool,
            tc.tile_pool(name="small", bufs=1) as smallpool,
            tc.tile_pool(name="psum", bufs=2, space=bass.MemorySpace.PSUM) as psumpool,
            tc.tile_pool(name="psumS", bufs=1, space=bass.MemorySpace.PSUM) as psSpool,
        ):
            # persistent SBUF: interleaved Gram store [b, c, layer] per row
            gbig = [
                gbpool.tile([P, WC, L], bf16, tag=f"Gbig{r}", name=f"Gbig{r}")
                for r in range(NR)
            ]
            # NOTE: matmul start=True pending-zeroes its whole PSUM bank, so
            # each bank may host only ONE accumulation group at a time (the
            # 512/512/128 Gram split below is bank-aligned for this reason,
            # and the S classes accumulate sequentially with copies between).
            QW = 4 * L          # S-stage stationary width (4 Gram columns)
            QN = 4 * LB         # S-stage moving width (B layers only)
            ptS = psSpool.tile([P, 3 * QN], f32, tag="psS")
            # VectorE A-block pair accumulators, (r, class)-sliced
            pairacc = smallpool.tile([P, NR * 3 * NPA], f32, tag="pairacc")
            sttout = smallpool.tile([P, WC], bf16, tag="sttout")

            for l in range(L):
                pt = [
                    psumpool.tile([P, WC], f32, tag="pm", name=f"pm{r}")
                    for r in range(NR)
                ]
                for jg in range(JT // JG):
                    rt = rtpool.tile([P, JG, 2, NXC * P], fp8, tag="rt")
                    nc.sync.dma_start(rt[:, :, :, :], xr[l, jg])
                    for jj in range(JG):
                        j = jg * JG + jj
                        for r in range(NR):
                            lhs = rt[:, jj, :, r * P : (r + 1) * P]
                            # 9 cyclic blocks = contiguous 1152 rhs columns,
                            # split 512/512/128 on PSUM bank boundaries
                            for c0, cw in ((0, 512), (512, 512), (1024, 128)):
                                nc.tensor.matmul(
                                    pt[r][:, c0 : c0 + cw],
                                    lhsT=lhs,
                                    rhs=rt[:, jj, :, r * P + c0 : r * P + c0 + cw],
                                    start=(j == 0),
                                    stop=(j == JT - 1),
                                    perf_mode=DR,
                                )
                for r in range(NR):
                    nc.scalar.copy(gbig[r][:, :, l], pt[r][:, :])
                # A-block pairs (i, l), i <= l < LA reduce on the otherwise
                # idle VectorE while the DMA-bound main loop continues; the
                # three weight classes accumulate into separate columns
                if l < LA:
                    for i in range(l + 1):
                        p = l * (l + 1) // 2 + i
                        for r in range(NR):
                            for cls, (lo, hi) in enumerate(
                                ((0, P), (P, 8 * P), (8 * P, WC))
                            ):
                                nc.vector.scalar_tensor_tensor(
                                    out=sttout[:, lo:hi],
                                    in0=gbig[r][:, lo:hi, i],
                                    scalar=1.0,
                                    in1=gbig[r][:, lo:hi, l],
                                    op0=Alu.mult,
                                    op1=Alu.mult,
                                    accum_out=pairacc[
                                        :, (r * 3 + cls) * NPA + p : (r * 3 + cls) * NPA + p + 1
                                    ],
                                )

            # partition-reduce the VectorE pair accumulators on idle GpSimd
            asum = smallpool.tile([P, NR * 3 * NPA], f32, tag="asum")
            nc.gpsimd.tensor_reduce(
                asum[0:1, :], pairacc[:, :], axis=mybir.AxisListType.C, op=Alu.add
            )
            nc.sync.dma_start(a_out[:, :], asum[0:1, :])

            # S-stage on TensorE: [128b, 96] x [128b, 96] matmuls, 4 Gram
            # columns per call (gbig's [c, l] layout is contiguous, so 4
            # columns = one flat 96-wide operand).  Only the 4 diagonal
            # [24, 24] blocks of each [96, 96] product matter; the host
            # discards the cross-column junk.  Weight classes accumulate
            # SEQUENTIALLY (one live group in the shared PSUM bank) and are
            # copied out before the next class's start=True re-marks the bank.
            # lhsT spans all L layers (out rows cover every i), rhs spans only
            # the LB "B" layers -- the A-block intra-pairs came from VectorE
            sall = smallpool.tile([P, 3 * QN], f32, tag="sall")
            cls_bounds = {0: (0, P), 1: (P, 8 * P), 2: (8 * P, WC)}
            for cls in range(3):
                lo, hi = cls_bounds[cls]
                for r in range(NR):
                    for c in range(lo, hi, 4):
                        nc.tensor.matmul(
                            ptS[0:QW, cls * QN : (cls + 1) * QN],
                            lhsT=gbig[r][:, c : c + 4, :],
                            rhs=gbig[r][:, c : c + 4, LA:L],
                            start=(r == 0 and c == lo),
                            stop=(r == NR - 1 and c == hi - 4),
                        )
                nc.scalar.copy(
                    sall[0:QW, cls * QN : (cls + 1) * QN],
                    ptS[0:QW, cls * QN : (cls + 1) * QN],
                )
            for g in range(3):
                nc.sync.dma_start(s_out[g], sall[0:QW, g * QN : (g + 1) * QN])

    nc.compile()
    _NC_CACHE["nc"] = nc
    return nc


def _run(activations, trace=False):
    from concourse.bass_utils import run_bass_kernel_spmd

    x = np.asarray(activations, dtype=np.float32)
    assert x.shape == (L, B, D)
    xt_np = np.ascontiguousarray(x.transpose(0, 2, 1)).astype(ml_dtypes.float8_e4m3)
    s_star = xt_np.astype(np.float64).sum(axis=2)  # [L, D], exact sum of fp8 X

    in_maps = []
    for c in range(NCORES):
        lo = NR * c * P
        rolled = np.concatenate([xt_np[:, :, lo:], xt_np[:, :, :lo]], axis=2)[
            :, :, : NXC * P
        ]
        # pack to the SBUF staging layout: [L, jg, p, jj, i, n] so each
        # (layer, jg) DMA is one fully contiguous block
        packed = np.ascontiguousarray(
            rolled.reshape(L, JT // JG, JG, 2, P, NXC * P).transpose(0, 1, 4, 2, 3, 5)
        )
        in_maps.append({"xr": packed})
    nc = _build()
    try:
        res = run_bass_kernel_spmd(
            nc, in_maps, core_ids=list(range(NCORES)), trace=trace
        )
    except Exception:
        # transient NRT_EXEC_UNIT_UNRECOVERABLE device states have been
        # observed to clear on the next attempt
        import time

        time.sleep(5)
        res = run_bass_kernel_spmd(
            nc, in_maps, core_ids=list(range(NCORES)), trace=trace
        )

    S = np.zeros((L, L), dtype=np.float64)
    for c in range(NCORES):
        # TensorE part: [3, 4*L, 4*LB] quad blocks, diagonal-in-quad only
        g = res.results[c]["s_out"].astype(np.float64).reshape(3, 4, L, 4, LB)
        gd = [sum(g[i, d, :, d, :] for d in range(4)) for i in range(3)]
        Sc = np.zeros((L, L))
        Sc[:, LA:] = gd[0] + 2.0 * gd[1] + gd[2]
        Sc[LA:, :LA] = Sc[:LA, LA:].T
        # VectorE part: A-block pairs, (r, class)-sliced partials
        a = res.results[c]["a_out"].astype(np.float64).reshape(NR, 3, NPA)
        av = a.sum(axis=0)
        pa = av[0] + 2.0 * av[1] + av[2]
        for l in range(LA):
            for i in range(l + 1):
                v = pa[l * (l + 1) // 2 + i]
                Sc[i, l] = v
                Sc[l, i] = v
        S += Sc

    # row-sum statistics are O(L*B*D) -- computed host-side on the same
    # quantized values the device consumed
    xq = xt_np.astype(np.float32)                  # [L, D, B]
    rowsum = np.einsum("ldb,ld->lb", xq, s_star.astype(np.float32))
    T = np.einsum("ib,jb->ij", rowsum, rowsum, dtype=np.float64)
    u = np.einsum("ld,ld->l", s_star, s_star)
    hsic = S - (2.0 / B) * T + np.outer(u, u) / (B * B)
    norms = np.sqrt(np.diagonal(hsic))
    cka = hsic / (norms[:, None] * norms[None, :])
    return cka.astype(np.float32), res


def kernel(activations):
    cka, _ = _run(activations, trace=False)
    return cka


def run_traced(activations):
    return _run(activations, trace=True)


# revision 39
# speedup vs baseline: 1.0476x; 1.0055x over previous
"""Linear-CKA map kernel for Trainium2 (8 NeuronCores, SPMD, no collectives).

Math: for activations X[l] ([B, D] per layer), the reference computes
Gram matrices G_l = X_l X_l^T, double-centers them (Gc = H G H), and
hsic[i,j] = <Gc_i, Gc_j>, cka = hsic / sqrt(diag outer).

We use the expansion (H idempotent, G symmetric):
    hsic_ij = S_ij - (2/B) * T_ij + u_i u_j / B^2
      S_ij = <G_i, G_j>
      rowsum_l[b] = sum_c G_l[b, c] = X_l[b, :] . s_l,  s_l = sum_b X_l[b, :]
      T_ij = sum_b rowsum_i[b] rowsum_j[b]
      u_l  = s_l . s_l

Sharding: the Gram is symmetric, so only its block upper triangle is
needed.  With 16x16 blocks of [128, 128], core k computes the cyclic
cover blocks (bi, (bi + t) mod 16), t = 0..8, for its two block rows
bi in {2k, 2k+1} -- a perfectly uniform SPMD program (18 blocks per
core; every unordered block pair is covered once, except the t=8
antipodal blocks which two cores split).  Weight classes: t=0 diag
blocks count once, t=1..7 twice, t=8 once; the three classes accumulate
into separate [24,24] PSUM groups and the host combines g0 + 2*g1 + g2.

Per-core input: xr = X^T columns [2k*128 : 2k*128+1280] (mod B), in
fp8-e4m3 (CKA is a normalized statistic, so input quantization noise
stays ~1e-4 in the output) -- both Gram matmul operands come from this
one 63 MB slice, read exactly once, and the matmuls run in DoubleRow
mode (256-deep contraction, 2 fp8 MACs/cell/cycle).  S itself reduces
on TensorE: the PSUM->SBUF copies interleave layers ([b, c, layer] in
bf16), then [128b, 96] x [128b, 96] matmuls (4 Gram columns per call)
accumulate S in PSUM; the host keeps the diagonal [24,24] blocks.
Partial S is the only device output, summed on the host -- no device
collective at all.  The O(L*B*D) row-sum statistics T and u are
computed on the host (0.02% of the FLOPs).
"""

import numpy as np
import ml_dtypes

L, B, D = 24, 2048, 2048
NCORES = 8
P = 128
NBLK = B // P               # 16 block rows/cols
JT = D // (2 * P)           # 8 double-row contraction tiles (256 deep each)
JG = 2                      # j-tiles fetched per rhs DMA
NT = 9                      # cyclic block offsets t = 0..8 per block row
NR = 2                      # block rows per core
NXC = NT + NR - 1           # 10 column chunks staged per core
WC = NT * P                 # 1152 Gram columns per block row
LA = 10                     # layers whose intra-pairs reduce on idle VectorE
LB = L - LA                 # layers handled by the TensorE S-stage rhs
NPA = LA * (LA + 1) // 2    # VectorE pair count

_NC_CACHE = {}


def _build():
    if "nc" in _NC_CACHE:
        return _NC_CACHE["nc"]
    import concourse.bass as bass
    from concourse import bacc, mybir, tile

    f32 = mybir.dt.float32
    bf16 = mybir.dt.bfloat16
    fp8 = mybir.dt.float8e4
    DR = mybir.MatmulPerfMode.DoubleRow
    Act = mybir.ActivationFunctionType
    Alu = mybir.AluOpType

    nc = bacc.Bacc("TRN2", target_bir_lowering=False, debug=False)

    # xr is host-packed to exactly match the SBUF staging tiles: one fully
    # linear [P, JG, 2, 1280] block per (layer, jg) DMA
    xr = nc.dram_tensor(
        "xr", [L, JT // JG, P, JG, 2, NXC * P], fp8, kind="ExternalInput"
    )
    s_out = nc.dram_tensor("s_out", [3, 4 * L, 4 * LB], f32, kind="ExternalOutput")
    a_out = nc.dram_tensor("a_out", [1, NR * 3 * NPA], f32, kind="ExternalOutput")

    with tile.TileContext(nc) as tc:
        with (
            tc.tile_pool(name="gb", bufs=1) as gbpool,
            tc.tile_pool(name="rt", bufs=5) as rtpool,
            tc.tile_pool(name="small", bufs=1) as smallpool,
            tc.tile_pool(name="psum", bufs=2, space=bass.MemorySpace.PSUM) as psumpool,
            tc.tile_pool(name="psumS", bufs=1, space=bass.MemorySpace.PSUM) as psSpool,
        ):
            # persistent SBUF: interleaved Gram store [b, c, layer] per row
            gbig = [
                gbpool.tile([P, WC, L], bf16, tag=f"Gbig{r}", name=f"Gbig{r}")
                for r in range(NR)
            ]
            # NOTE: matmul start=True pending-zeroes its whole PSUM bank, so
            # each bank may host only ONE accumulation group at a time (the
            # 512/512/128 Gram split below is bank-aligned for this reason,
            # and the S classes accumulate sequentially with copies between).
            QW = 4 * L          # S-stage stationary width (4 Gram columns)
            QN = 4 * LB         # S-stage moving width (B layers only)
            ptS = psSpool.tile([P, 3 * QN], f32, tag="psS")
            # VectorE A-block pair accumulators, (r, class)-sliced
            pairacc = smallpool.tile([P, NR * 3 * NPA], f32, tag="pairacc")
            sttout = smallpool.tile([P, WC], bf16, tag="sttout")

            for l in range(L):
                pt = [
                    psumpool.tile([P, WC], f32, tag="pm", name=f"pm{r}")
                    for r in range(NR)
                ]
                for jg in range(JT // JG):
                    rt = rtpool.tile([P, JG, 2, NXC * P], fp8, tag="rt")
                    nc.sync.dma_start(rt[:, :, :, :], xr[l, jg])
                    for jj in range(JG):
                        j = jg * JG + jj
                        for r in range(NR):
                            lhs = rt[:, jj, :, r * P : (r + 1) * P]
                            # 9 cyclic blocks = contiguous 1152 rhs columns,
                            # split 512/512/128 on PSUM bank boundaries
                            for c0, cw in ((0, 512), (512, 512), (1024, 128)):
                                nc.tensor.matmul(
                                    pt[r][:, c0 : c0 + cw],
                                    lhsT=lhs,
                                    rhs=rt[:, jj, :, r * P + c0 : r * P + c0 + cw],
                                    start=(j == 0),
                                    stop=(j == JT - 1),
                                    perf_mode=DR,
                                )
                for r in range(NR):
                    nc.scalar.copy(gbig[r][:, :, l], pt[r][:, :])
                # A-block pairs (i, l), i <= l < LA reduce on the otherwise
                # idle VectorE while the DMA-bound main loop continues; the
                # three weight classes accumulate into separate columns
                if l < LA:
                    for i in range(l + 1):
                        p = l * (l + 1) // 2 + i
                        for r in range(NR):
                            for cls, (lo, hi) in enumerate(
                                ((0, P), (P, 8 * P), (8 * P, WC))
                            ):
                                nc.vector.scalar_tensor_tensor(
                                    out=sttout[:, lo:hi],
                                    in0=gbig[r][:, lo:hi, i],
                                    scalar=1.0,
                                    in1=gbig[r][:, lo:hi, l],
                                    op0=Alu.mult,
                                    op1=Alu.mult,
                                    accum_out=pairacc[
                                        :, (r * 3 + cls) * NPA + p : (r * 3 + cls) * NPA + p + 1
                                    ],
                                )

            # partition-reduce the VectorE pair accumulators on idle GpSimd
            asum = smallpool.tile([P, NR * 3 * NPA], f32, tag="asum")
            nc.gpsimd.tensor_reduce(
                asum[0:1, :], pairacc[:, :], axis=mybir.AxisListType.C, op=Alu.add
            )
            nc.sync.dma_start(a_out[:, :], asum[0:1, :])

            # S-stage on TensorE: [128b, 96] x [128b, 96] matmuls, 4 Gram
            # columns per call (gbig's [c, l] layout is contiguous, so 4
            # columns = one flat 96-wide operand).  Only the 4 diagonal
            # [24, 24] blocks of each [96, 96] product matter; the host
            # discards the cross-column junk.  Weight classes accumulate
            # SEQUENTIALLY (one live group in the shared PSUM bank) and are
            # copied out before the next class's start=True re-marks the bank.
            # lhsT spans all L layers (out rows cover every i), rhs spans only
            # the LB "B" layers -- the A-block intra-pairs came from VectorE
            sall = smallpool.tile([P, 3 * QN], f32, tag="sall")
            cls_bounds = {0: (0, P), 1: (P, 8 * P), 2: (8 * P, WC)}
            for cls in range(3):
                lo, hi = cls_bounds[cls]
                for r in range(NR):
                    for c in range(lo, hi, 4):
                        nc.tensor.matmul(
                            ptS[0:QW, cls * QN : (cls + 1) * QN],
                            lhsT=gbig[r][:, c : c + 4, :],
                            rhs=gbig[r][:, c : c + 4, LA:L],
                            start=(r == 0 and c == lo),
                            stop=(r == NR - 1 and c == hi - 4),
                        )
                nc.scalar.copy(
                    sall[0:QW, cls * QN : (cls + 1) * QN],
                    ptS[0:QW, cls * QN : (cls + 1) * QN],
                )
            for g in range(3):
                nc.sync.dma_start(s_out[g], sall[0:QW, g * QN : (g + 1) * QN])

    nc.compile()
    _NC_CACHE["nc"] = nc
    return nc


def _run(activations, trace=False):
    from concourse.bass_utils import run_bass_kernel_spmd

    x = np.asarray(activations, dtype=np.float32)
    assert x.shape == (L, B, D)
    xt_np = np.ascontiguousarray(x.transpose(0, 2, 1)).astype(ml_dtypes.float8_e4m3)
    s_star = xt_np.astype(np.float64).sum(axis=2)  # [L, D], exact sum of fp8 X

    in_maps = []
    for c in range(NCORES):
        lo = NR * c * P
        rolled = np.concatenate([xt_np[:, :, lo:], xt_np[:, :, :lo]], axis=2)[
            :, :, : NXC * P
        ]
        # pack to the SBUF staging layout: [L, jg, p, jj, i, n] so each
        # (layer, jg) DMA is one fully contiguous block
        packed = np.ascontiguousarray(
            rolled.reshape(L, JT // JG, JG, 2, P, NXC * P).transpose(0, 1, 4, 2, 3, 5)
        )
        in_maps.append({"xr": packed})
    nc = _build()
    try:
        res = run_bass_kernel_spmd(
            nc, in_maps, core_ids=list(range(NCORES)), trace=trace
        )
    except Exception:
        # transient NRT_EXEC_UNIT_UNRECOVERABLE device states have been
        # observed to clear on the next attempt
        import time

        time.sleep(5)
        res = run_bass_kernel_spmd(
            nc, in_maps, core_ids=list(range(NCORES)), trace=trace
        )

    S = np.zeros((L, L), dtype=np.float64)
    for c in range(NCORES):
        # TensorE part: [3, 4*L, 4*LB] quad blocks, diagonal-in-quad only
        g = res.results[c]["s_out"].astype(np.float64).reshape(3, 4, L, 4, LB)
        gd = [sum(g[i, d, :, d, :] for d in range(4)) for i in range(3)]
        Sc = np.zeros((L, L))
        Sc[:, LA:] = gd[0] + 2.0 * gd[1] + gd[2]
        Sc[LA:, :LA] = Sc[:LA, LA:].T
        # VectorE part: A-block pairs, (r, class)-sliced partials
        a = res.results[c]["a_out"].astype(np.float64).reshape(NR, 3, NPA)
        av = a.sum(axis=0)
        pa = av[0] + 2.0 * av[1] + av[2]
        for l in range(LA):
            for i in range(l + 1):
                v = pa[l * (l + 1) // 2 + i]
                Sc[i, l] = v
                Sc[l, i] = v
        S += Sc

    # row-sum statistics are O(L*B*D) -- computed host-side on the same
    # quantized values the device consumed
    xq = xt_np.astype(np.float32)                  # [L, D, B]
    rowsum = np.einsum("ldb,ld->lb", xq, s_star.astype(np.float32))
    T = np.einsum("ib,jb->ij", rowsum, rowsum, dtype=np.float64)
    u = np.einsum("ld,ld->l", s_star, s_star)
    hsic = S - (2.0 / B) * T + np.outer(u, u) / (B * B)
    norms = np.sqrt(np.diagonal(hsic))
    cka = hsic / (norms[:, None] * norms[None, :])
    return cka.astype(np.float32), res


def kernel(activations):
    cka, _ = _run(activations, trace=False)
    return cka


def run_traced(activations):
    return _run(activations, trace=True)
